# revision 10
# baseline (speedup 1.0000x reference)
"""Causal self-attention (B=2, T=2048, C=1024, H=16) on 8 TRN2 NeuronCores.

Megatron-style tensor parallelism over heads: each core computes 2 of the 16
heads (Wq/Wk/Wv column-sharded, Wo row-sharded) and produces a partial output
projection; the partials are summed on the host (the all-reduce).

Per-core device dataflow (PE contraction dim is always the partition dim):
  xt  [128, tb*(ct,t)] host-pretiled so each t-block load is a flat 2D DMA
  QT/KT = W.T @ x (K-tiled over C), bias+cast fused on the scalar engine
  V tiles = PE-transpose of VT with per-head ones columns (memset once,
            strided); transpose outputs copied with one strided DVE op
  S^T  = K_loc @ Q_loc^T per (batch, head, 128-j-tile, 512-i-block);
         diagonal j-tiles only compute the causally live i-range
  P^T  = exp(S^T/8) on ACT (trimmed 3D AP); mask-multiply on diagonal tiles
  O^T|s = [V|1].T @ P^T accumulated over j (ones row gives softmax sums);
          diagonal j-tiles processed FIRST so start/stop cover the full block
  yloc = O^T * (1/s): reciprocal (DVE, bf16), broadcast via K=1 PE matmuls,
         per-head DVE multiplies writing both partition halves of ylocT
  yT_part = Wo_loc^T @ ylocT -> DRAM [C, B*T] (casts via DVE)
Host: y = (sum_cores yT_part).T + bo, reshape to [B, T, C].

Emission order fills the PE: qkv(0); [att(0,i) || qkv(i+1)] ...; att(1,3)
so attention (ACT-heavy) always overlaps projection work (PE-heavy).
"""

import sys

if "/opt/trn_rl_repo" not in sys.path:
    sys.path.insert(0, "/opt/trn_rl_repo")

import numpy as np

import concourse.bass as bass
import concourse.tile as tile
from concourse import bacc
from concourse import mybir
from concourse.bass_utils import run_bass_kernel_spmd

F32 = mybir.dt.float32
BF16 = mybir.dt.bfloat16
AF = mybir.ActivationFunctionType
ALU = mybir.AluOpType

B, T, C, H = 2, 2048, 1024, 16
D = C // H          # 64
NCORES = 8
HL = H // NCORES    # 2 local heads
CL = C // NCORES    # 128 local channels
BT = B * T          # 4096
TB = 512            # t-block (matmul moving width, i-block size)
NTB = BT // TB      # 8 token blocks
NKT = C // 128      # 8 contraction tiles for projections
IB = T // TB        # 4 i-blocks per batch
NJT = T // 128      # 16 j-tiles per batch
VW = 130            # V tile width: 2 heads x (64 V cols + 1 ones col)


def build_nc() -> bass.Bass:
    nc = bacc.Bacc()

    xt_d = nc.declare_dram_parameter("xt", [128, NTB * 4096], BF16, isOutput=False)
    wq_d = nc.declare_dram_parameter("wqT", [128, C], BF16, isOutput=False)
    wk_d = nc.declare_dram_parameter("wkT", [128, C], BF16, isOutput=False)
    wv_d = nc.declare_dram_parameter("wvT", [128, C], BF16, isOutput=False)
    wo_d = nc.declare_dram_parameter("woT", [CL, C], BF16, isOutput=False)
    bqkv_d = nc.declare_dram_parameter("bqkv", [CL, 3], F32, isOutput=False)
    mask_d = nc.declare_dram_parameter("masks", [128, 4 * 1024], BF16, isOutput=False)
    id_d = nc.declare_dram_parameter("ident", [128, 128], BF16, isOutput=False)
    yT_d = nc.declare_dram_parameter("yT", [C, BT], BF16, isOutput=True)

    with tile.TileContext(nc) as tc:
        with (
            tc.tile_pool(name="const", bufs=1) as const,
            tc.tile_pool(name="work", bufs=2) as work,
            tc.tile_pool(name="psum", bufs=2, space="PSUM") as psum,
        ):
            # ---------------- constants / persistent state ----------------
            wq_sb = const.tile([128, C], BF16)
            wk_sb = const.tile([128, C], BF16)
            wv_sb = const.tile([128, C], BF16)
            xt_sb = const.tile([128, NTB * 4096], BF16)
            id_sb = const.tile([128, 128], BF16)
            bqkv_sb = const.tile([128, 3], F32)
            wo_sb = const.tile([128, C], BF16)
            mask_sb = const.tile([128, 4 * 1024], BF16)

            # DMA priority order: first projections' weights, then x tile 0,
            # then the rest; wo/mask are not needed until attention starts.
            nc.sync.dma_start(wq_sb[:, :], wq_d[:, :])
            nc.sync.dma_start(wk_sb[:, :], wk_d[:, :])
            nc.sync.dma_start(wv_sb[:, :], wv_d[:, :])
            for half in range(2):
                s = slice(half * 2048, half * 2048 + 2048)
                nc.sync.dma_start(xt_sb[:, s], xt_d[:, s])
            nc.sync.dma_start(id_sb[:, :], id_d[:, :])
            nc.sync.dma_start(bqkv_sb[:, :], bqkv_d[:, :])
            for tb in range(1, 4):
                for half in range(2):
                    s = slice(tb * 4096 + half * 2048, tb * 4096 + half * 2048 + 2048)
                    nc.sync.dma_start(xt_sb[:, s], xt_d[:, s])
            nc.sync.dma_start(wo_sb[:, :], wo_d[:, :])
            nc.sync.dma_start(mask_sb[:, :], mask_d[:, :])
            for tb in range(4, NTB):
                for half in range(2):
                    s = slice(tb * 4096 + half * 2048, tb * 4096 + half * 2048 + 2048)
                    nc.sync.dma_start(xt_sb[:, s], xt_d[:, s])

            QT = const.tile([128, BT], BF16)
            KT = const.tile([128, BT], BF16)
            ylocT = const.tile([128, BT], BF16)
            V = const.tile([128, (BT // 128) * VW], BF16)
            # per-head ones columns of V: cols g*130 + {64, 129}
            # (bf16 1.0 = 0x3F80 via bitcast; float memset on bf16 APs and
            # strided memsets are not reliable on hardware)
            for _jg in range(BT // 128):
                for _c in (_jg * VW + 64, _jg * VW + 129):
                    nc.gpsimd.memset(V[:, _c:_c + 1].bitcast(mybir.dt.uint16),
                                     0x3F80)

            # ---------------- phase 1: Q/K/V projections -------------------
            # Emitted as small chunks so projection / output-projection
            # matmuls can be woven between attention j-tiles: the PE then
            # always has dependency-free work, stays continuously busy, and
            # holds its fast p-state through the attention phases.
            def qkv_chunks(tb):
                tcols = slice(tb * TB, (tb + 1) * TB)
                xt = xt_sb[:, tb * 4096:(tb + 1) * 4096]
                state = {}

                def proj_part(wi, w_sb, dst, lo, hi):
                    def emit():
                        if lo == 0:
                            state[wi] = psum.tile([128, TB], F32, tag="mm",
                                                  name=f"ps_{wi}_{tb}")
                        ps = state[wi]
                        for ct in range(lo, hi):
                            nc.tensor.matmul(
                                ps[:, :],
                                w_sb[:, ct * 128:(ct + 1) * 128],
                                xt[:, ct * TB:(ct + 1) * TB],
                                start=(ct == 0), stop=(ct == NKT - 1),
                            )
                        if hi == NKT:
                            if dst is not None:
                                nc.scalar.activation(
                                    dst[:, tcols], ps[:, :], AF.Identity,
                                    bias=bqkv_sb[:, wi:wi + 1])
                            else:
                                vt = work.tile([128, TB], BF16, tag="vtsb",
                                               name=f"vt_{tb}")
                                state["vt"] = vt
                                nc.scalar.activation(
                                    vt[:, :], ps[:, :], AF.Identity,
                                    bias=bqkv_sb[:, 2:3])
                    return emit

                def tr_part(q):
                    def emit():
                        jg = tb * 4 + q
                        tp = psum.tile([128, 128], BF16, tag="mm",
                                       name=f"tp_{jg}")
                        nc.tensor.transpose(
                            tp[:, :], state["vt"][:, q * 128:(q + 1) * 128],
                            id_sb[:, :])
                        off = jg * VW
                        vdst = V[:, off:off + VW].rearrange(
                            "p (h z) -> p h z", h=2)[:, :, 0:64]
                        vsrc = tp[:, :].rearrange("p (h z) -> p h z", h=2)
                        nc.vector.tensor_copy(vdst, vsrc)
                    return emit

                chunks = []
                for wi, (w_sb, dst) in enumerate(
                    ((wq_sb, QT), (wk_sb, KT), (wv_sb, None))
                ):
                    chunks.append(proj_part(wi, w_sb, dst, 0, 4))
                    chunks.append(proj_part(wi, w_sb, dst, 4, 8))
                for q in range(4):
                    chunks.append(tr_part(q))
                return chunks

            def outproj_chunks(b, ib, use_act=False):
                i0 = b * T + ib * TB
                bcols = slice(i0, i0 + TB)

                def co_part(co):
                    def emit():
                        yp = psum.tile([128, TB], F32, tag="mm",
                                       name=f"yp_{b}_{ib}_{co}")
                        nc.tensor.matmul(
                            yp[:, :],
                            wo_sb[:, co * 128:(co + 1) * 128],
                            ylocT[:, bcols],
                            start=True, stop=True,
                        )
                        yo = work.tile([128, TB], BF16, tag="yo", bufs=4,
                                       name=f"yo_{b}_{ib}_{co}")
                        if use_act and co % 2 == 0:
                            nc.scalar.copy(yo[:, :], yp[:, :])
                        else:
                            nc.vector.tensor_copy(yo[:, :], yp[:, :])
                        nc.sync.dma_start(
                            yT_d[co * 128:(co + 1) * 128, bcols], yo[:, :])
                    return emit

                return [co_part(co) for co in range(8)]

            # ------- phase 2+3: attention per i-block, weaving in fillers ----
            def emit_att_core(b, ib, fillers=()):
                fillers = list(fillers)
                done = 0
                i0 = b * T + ib * TB
                njt = 4 * (ib + 1)
                ots = [
                    psum.tile([65, TB], F32, tag="ot", name=f"ot_{b}_{ib}_{h}")
                    for h in range(HL)
                ]
                for n in range(njt):
                    jt = n
                    jg = b * NJT + jt
                    q = jt - (njt - 4)  # diag index 0..3, negative if full
                    z0 = 128 * q if q > 0 else 0
                    icols = slice(i0 + z0, i0 + TB)
                    st = psum.tile([128, 2 * TB], F32, tag="st",
                                   name=f"st_{b}_{ib}_{jt}")
                    for h in range(HL):
                        hs = slice(h * D, (h + 1) * D)
                        nc.tensor.matmul(
                            st[:, h * TB + z0:(h + 1) * TB],
                            KT[hs, jg * 128:(jg + 1) * 128],
                            QT[hs, icols],
                            start=True, stop=True,
                        )
                    pt = work.tile([128, 2 * TB], BF16, tag="pt", bufs=6,
                                   name=f"pt_{b}_{ib}_{jt}")
                    stv = st[:, :].rearrange("p (h z) -> p h z", h=2)[:, :, z0:]
                    ptv = pt[:, :].rearrange("p (h z) -> p h z", h=2)[:, :, z0:]
                    nc.scalar.activation(ptv, stv, AF.Exp, scale=0.125)
                    if q >= 0:
                        mv = mask_sb[:, q * 1024:(q + 1) * 1024].rearrange(
                            "p (h z) -> p h z", h=2)[:, :, z0:]
                        nc.vector.tensor_tensor(ptv, ptv, mv, ALU.mult)
                    for h in range(HL):
                        off = jg * VW + h * 65
                        nc.tensor.matmul(
                            ots[h][:, z0:],
                            V[:, off:off + 65],
                            pt[:, h * TB + z0:(h + 1) * TB],
                            start=(n == 0), stop=(n == njt - 1),
                        )
                    want = len(fillers) * (n + 1) // njt
                    while done < want:
                        fillers[done]()
                        done += 1
                while done < len(fillers):
                    fillers[done]()
                    done += 1
                # normalization chain: s -> 1/s -> broadcast -> scale
                bcols = slice(i0, i0 + TB)
                # copy s rows to SBUF partition 0 first: reciprocal_approx_fast
                # straight off PSUM at partition base 64 breaks on hardware
                s_sb = work.tile([1, 2 * TB], F32, tag="s", name=f"s_{b}_{ib}")
                rb = work.tile([1, 2 * TB], F32, tag="r", name=f"r_{b}_{ib}")
                for h in range(HL):
                    nc.vector.tensor_copy(s_sb[0:1, h * TB:(h + 1) * TB],
                                          ots[h][64:65, :])
                    nc.vector.reciprocal_approx_fast(
                        rb[0:1, h * TB:(h + 1) * TB],
                        s_sb[0:1, h * TB:(h + 1) * TB])
                bcs = []
                for h in range(HL):
                    bc = work.tile([64, TB], F32, tag="bc",
                                   name=f"bc_{b}_{ib}_{h}")
                    nc.gpsimd.partition_broadcast(
                        bc[:, :], rb[0:1, h * TB:(h + 1) * TB])
                    bcs.append(bc)
                # h0 writes its partition range directly; h1 needs a partition
                # shift (DVE out base must match in base on HW) -> SBUF DMA
                nc.vector.tensor_tensor(
                    ylocT[0:64, bcols], ots[0][0:64, :], bcs[0][:, :], ALU.mult)
                yn1 = work.tile([64, TB], BF16, tag="yn1", name=f"yn_{b}_{ib}")
                nc.vector.tensor_tensor(
                    yn1[:, :], ots[1][0:64, :], bcs[1][:, :], ALU.mult)
                nc.sync.dma_start(ylocT[64:128, bcols], yn1[:, :])

            for ch in qkv_chunks(0):
                ch()
            # Pairing invariant: a block's own projections (its Q columns and
            # its diagonal j-tiles' K/V) must be FULLY emitted in an earlier
            # segment — a same-segment filler write after the reading matmul
            # becomes a write-after-read ordering and the read sees garbage.
            atts = [(0, 0), (0, 1), (0, 2), (0, 3), (1, 1), (1, 2), (1, 3),
                    (1, 0)]
            qkv_per_seg = [[1], [2], [3], [4, 5], [6], [7], [], []]
            prev = None
            for seg in range(8):
                qc = []
                for tb in qkv_per_seg[seg]:
                    qc += qkv_chunks(tb)
                oc = ([] if prev is None else
                      outproj_chunks(*prev, use_act=(seg == 7)))
                # round-robin merge: qkv feeds later segments, out-proj
                # drains the previous one
                fillers = []
                for i in range(max(len(qc), len(oc))):
                    if i < len(qc):
                        fillers.append(qc[i])
                    if i < len(oc):
                        fillers.append(oc[i])
                emit_att_core(*atts[seg], fillers=fillers)
                prev = atts[seg]
            for ch in outproj_chunks(*prev, use_act=True):
                ch()
    nc.compile()
    return nc


def _host_inputs(x, Wq, bq, Wk, bk, Wv, bv, Wo):
    """Build the 8 per-core input maps (host-side layout prep + sharding)."""
    import ml_dtypes
    bf16 = ml_dtypes.bfloat16
    xT = np.ascontiguousarray(x.reshape(BT, C).T.astype(bf16))  # [C, BT]
    # xt[p, tb*4096 + ct*512 + t] = xT[ct*128+p, tb*512+t]
    xt = np.ascontiguousarray(
        xT.reshape(NKT, 128, NTB, TB).transpose(1, 2, 0, 3).reshape(128, NTB * 4096))
    masks = np.zeros((128, 4 * 1024), "float32")
    jj = np.arange(128, dtype=np.int32)[:, None]
    ii = np.arange(TB, dtype=np.int32)[None, :]
    for q in range(4):
        m = (ii >= 128 * q + jj).astype(np.float32)
        masks[:, q * 1024:q * 1024 + TB] = m
        masks[:, q * 1024 + TB:(q + 1) * 1024] = m
    masks = masks.astype(bf16)
    ident = np.eye(128, dtype=bf16)

    def wtile(W, rows):
        # device layout: w_sb[p, k*128 + j] = W[rows][j, k*128 + p]
        wT = W[rows, :].T.astype(bf16)                # [C, CL]
        return np.ascontiguousarray(
            wT.reshape(NKT, 128, CL).transpose(1, 0, 2).reshape(128, NKT * CL))

    in_maps = []
    for core in range(NCORES):
        rows = slice(core * CL, (core + 1) * CL)
        bqkv = np.stack([bq[rows], bk[rows], bv[rows]], axis=1).astype(np.float32)
        in_maps.append({
            "xt": xt,
            "wqT": wtile(Wq, rows),
            "wkT": wtile(Wk, rows),
            "wvT": wtile(Wv, rows),
            "woT": np.ascontiguousarray(Wo[:, rows].T.astype(bf16)),
            "bqkv": np.ascontiguousarray(bqkv),
            "masks": masks,
            "ident": ident,
        })
    return in_maps


_NC_CACHE = None


def _get_nc():
    global _NC_CACHE
    if _NC_CACHE is None:
        _NC_CACHE = build_nc()
    return _NC_CACHE


def _run(inputs, trace=False):
    x = np.asarray(inputs["x"], np.float32)
    in_maps = _host_inputs(
        x,
        np.asarray(inputs["Wq"], np.float32), np.asarray(inputs["bq"], np.float32),
        np.asarray(inputs["Wk"], np.float32), np.asarray(inputs["bk"], np.float32),
        np.asarray(inputs["Wv"], np.float32), np.asarray(inputs["bv"], np.float32),
        np.asarray(inputs["Wo"], np.float32),
    )
    res = run_bass_kernel_spmd(_get_nc(), in_maps, list(range(NCORES)), trace=trace)
    yT = np.zeros((C, BT), np.float64)
    for core in range(NCORES):
        yT += res.results[core]["yT"].astype(np.float64)
    y = yT.T.astype(np.float32) + np.asarray(inputs["bo"], np.float32)
    return y.reshape(B, T, C), res


def kernel(**inputs) -> np.ndarray:
    out, _ = _run(inputs, trace=False)
    return out


def _install_profile_hook():
    """Register the axon NTFF profile hook (the agent image ships the ctypes
    shim in trn_agent_boot but lacks the antenv.axon_hooks module)."""
    import types

    if "antenv.axon_hooks" in sys.modules:
        return
    sys.path.insert(0, "/root/.axon_site")
    from trn_agent_boot.trn_boot import _ntff_profile_via_ctypes

    mod = types.ModuleType("antenv.axon_hooks")
    hook = _ntff_profile_via_ctypes("/opt/axon/libaxon_pjrt.so")
    mod.get_axon_ntff_profile_hook = lambda: hook
    mod.set_axon_ntff_profile_hook = lambda h: None
    sys.modules["antenv.axon_hooks"] = mod
    import antenv

    antenv.axon_hooks = mod
    from concourse import bass_utils as _bu

    _bu.upload_artifacts = lambda tmpdir: tmpdir  # keep artifacts local


def kernel_profiled(**inputs):
    """Returns (output, exec_time_ns) using the NTFF profile of core 0."""
    _install_profile_hook()
    out, res = _run(inputs, trace=True)
    return out, res.exec_time_ns
